# revision 48
# baseline (speedup 1.0000x reference)
"""AWB loss (segment-reduce over softmax stats) on 8 Trainium2 NeuronCores.

Strategy (data-parallel over N, class-sorted sharding):
  * Host shards rows across the 8 cores AFTER stably sorting row indices by
    target class, padding each class run to 320-row blocks (16 partitions x
    20 row-slots, one class per block).  Sorting/padding is pure index
    metadata -- the heavy O(N*C) math all happens on-device.  Logits ship
    as bf16 (halves the HBM-bound stream; exp upconverts on read).
  * Device, per segment of [128 partitions x w slots x 100 classes]
    (segments are 80 slots mid-stream, 20/40 at the edges to shorten
    pipeline fill/drain):
      - DMA logits segment (contiguous per-partition lines)
      - ScalarE: E = exp(logits)  (bf16 -- enables 2x-rate DVE folds)
      - VectorE: three bf16 pair-folds 100->50->25->13 at 2x, then a
        13-wide 1x reduce_sum -> sumexp (f32).  ~1.6x faster than
        reducing the 100-wide f32 tile directly.
      - GPSIMD ap_gather: e_t = E[row, target_row]  (per-16-partition-group
        indices are legal because each group is single-class)
  * Per chunk of 4 matmul-tiles (packed wide DVE ops over [128, 320]):
      - parity-select the gathered bf16 pair -> e_t
      - r = 1/sumexp (fast approx);  pt = e_t * r;  pt2 = pt*pt
      - lg = ln(pt + 1e-6)   (exp+ln share one ACT table set via the
        activation-table patch below -- no table ping-pong)
      - per-block partial sums over the 20 row-slots
      - TensorE [128,12] x [128,8] block-id matmuls -> per-block
        (sum pt, sum pt^2, sum ln(pt+eps)) into PSUM
  * Host: per-block partials -> per-class sums (f64); pad rows (all
    duplicates of global row 0) are subtracted with a device-mimicked
    pt(row0) so the device needs no validity masking at all; then the
    tiny O(C) epilogue (Alpha, means, stds, softmax, final scalar).
"""

import math

import ml_dtypes
import numpy as np

P = 128          # SBUF partitions
C = 100          # classes
CH = C // 2      # folded class width
PB = 16          # partitions per block (one GPSIMD core group)
GB = 20          # row-slots per block
BLOCK = PB * GB  # 320 rows, single class
NQ = P // PB     # 8 partition-groups per tile
NGB = 4          # gb-groups per matmul-tile
GT = NGB * GB    # 80 row-slots per partition per matmul-tile
BPT = NQ * NGB   # 32 blocks per matmul-tile
TILE_ROWS = P * GT  # 10240 rows per matmul-tile
CORES = 8
EPS = 1e-6
LOGITS_BF16 = True

_GRAPH_CACHE = {}


def _segments(T):
    """Per-core g-slot segment widths (all multiples of GB, none crossing an
    80-slot matmul-tile boundary).  Short edge segments shorten pipeline
    fill and drain."""
    if T < 3:
        return [GT] * T
    return [GB, GB, 2 * GB] + [GT] * (T - 2) + [2 * GB, GB, GB]


def _seg_icols(w):
    """ap_gather wrapped-index columns for a segment of w indices."""
    return (w + PB - 1) // PB


def _patch_act_tables():
    """Make Exp and Ln resolve to the one table set that holds both
    (`natural_log_exp_and_others`), so the per-tile exp / per-chunk ln mix
    doesn't thrash ACT_TABLE_LOAD.  Only membership is edited -- set ids
    (list positions) are unchanged."""
    import functools

    import concourse.bacc as bacc_mod
    from concourse import mybir

    if getattr(bacc_mod, "_awb_act_patch", False):
        return
    orig = bacc_mod.get_activation_tables
    both = {mybir.ActivationFunctionType.Exp, mybir.ActivationFunctionType.Ln}
    combo = "natural_log_exp_and_others"

    @functools.cache
    def patched(arch):
        t = dict(orig(arch))
        if combo in t:
            t = {name: (set(fns) if name == combo else set(fns) - both)
                 for name, fns in t.items()}
        return t

    bacc_mod.get_activation_tables = patched
    bacc_mod._awb_act_patch = True


def _build_graph(T):
    """Build + compile the single-core Bass graph for T matmul-tiles (same
    NEFF on all 8 cores, SPMD with per-core inputs)."""
    if T in _GRAPH_CACHE:
        return _GRAPH_CACHE[T]

    from contextlib import ExitStack

    import concourse.bacc as bacc
    import concourse.tile as tile
    from concourse import mybir

    _patch_act_tables()

    f32 = mybir.dt.float32
    lg_dt = mybir.dt.bfloat16 if LOGITS_BF16 else f32
    i16 = mybir.dt.int16
    X = mybir.AxisListType.X

    segs = _segments(T)
    G_ALL = T * GT
    NC_TOT = sum(_seg_icols(w) for w in segs)

    nc = bacc.Bacc("TRN2", target_bir_lowering=False, debug=False,
                   num_devices=CORES)

    lg_d = nc.dram_tensor("logits", [P, G_ALL * C], lg_dt, kind="ExternalInput").ap()
    pr_d = nc.dram_tensor("parity", [P, G_ALL], mybir.dt.uint8,
                          kind="ExternalInput").ap()
    ic_d = nc.dram_tensor("icidx", [P, NC_TOT], i16, kind="ExternalInput").ap()
    bid_d = nc.dram_tensor("blockid", [P, NQ], f32, kind="ExternalInput").ap()
    out_d = nc.dram_tensor("out", [12, T * NQ], f32, kind="ExternalOutput").ap()

    bf16 = mybir.dt.bfloat16

    with tile.TileContext(nc) as tc, ExitStack() as ctx:
        # one pool for logits->exp: the exp runs in place (bf16 in, bf16
        # out, elementwise stream — safe), halving the big-tile footprint
        # and allowing 6 tiles in flight
        xp = ctx.enter_context(tc.tile_pool(name="xe", bufs=6))
        hpool = ctx.enter_context(tc.tile_pool(name="fold", bufs=2))
        pk = ctx.enter_context(tc.tile_pool(name="packed", bufs=1))
        pp = ctx.enter_context(tc.tile_pool(name="psum", bufs=1, space="PSUM"))

        # small input DMAs go on the (otherwise idle) GPSIMD SWDGE queue so
        # neither the SP logits stream nor the first ACT exp is delayed
        bid_sb = pk.tile([P, NQ], f32)
        nc.gpsimd.dma_start(out=bid_sb[:], in_=bid_d)
        eps_sb = pk.tile([P, 1], f32)
        nc.vector.memset(eps_sb[:], EPS)
        PAR = pk.tile([P, G_ALL], mybir.dt.uint8)
        nc.gpsimd.dma_start(out=PAR[:], in_=pr_d)
        ICI = pk.tile([P, NC_TOT], i16)
        nc.gpsimd.dma_start(out=ICI[:], in_=ic_d)

        SE = pk.tile([P, G_ALL], f32)
        # bf16 ap_gather works on element PAIRS (4-byte granularity): ETP
        # holds the gathered pair, the parity select below extracts the
        # target half.  +PB slack: per-segment gathers run in 16-index
        # groups, so edge segments overshoot; later segments overwrite the
        # pad garbage.
        ETP = pk.tile([P, G_ALL + PB, 2], bf16)
        ET = pk.tile([P, G_ALL], f32)
        R = pk.tile([P, G_ALL], f32)
        PT = pk.tile([P, G_ALL], f32)
        PT2 = pk.tile([P, G_ALL], f32)
        LG = pk.tile([P, G_ALL], f32)
        BS = pk.tile([P, T, 3, NGB], f32)
        psum = pp.tile([12, T * NQ], f32)

        def chunk_smalls(glo, ghi):
            sl = slice(glo, ghi)
            nc.vector.select(ET[:, sl], PAR[:, sl],
                             ETP[:, sl, 1], ETP[:, sl, 0])
            nc.vector.reciprocal_approx_fast(R[:, sl], SE[:, sl])
            nc.vector.tensor_mul(PT[:, sl], ET[:, sl], R[:, sl])
            nc.vector.tensor_mul(PT2[:, sl], PT[:, sl], PT[:, sl])
            nc.scalar.activation(LG[:, sl], PT[:, sl],
                                 mybir.ActivationFunctionType.Ln, bias=eps_sb[:])

        def chunk_mm(tlo, thi):
            sl = slice(tlo * GT, thi * GT)
            for v, buf in enumerate((PT, PT2, LG)):
                nc.vector.reduce_sum(
                    BS[:, tlo:thi, v, :],
                    buf[:, sl].rearrange("p (t gb j) -> p t gb j", gb=NGB, j=GB),
                    axis=X,
                )
            for t in range(tlo, thi):
                nc.tensor.matmul(
                    psum[:, t * NQ:(t + 1) * NQ],
                    BS[:, t, :, :], bid_sb[:],
                    start=True, stop=True,
                )

        g = 0
        icol = 0
        g_small_done = 0
        t_mm_done = 0
        for w in segs:
            E = xp.tile([P, w, C], lg_dt, tag="L")
            nc.sync.dma_start(
                out=E[:],
                in_=lg_d.rearrange("p (g c) -> p g c", c=C)[:, g:g + w, :])
            nc.scalar.activation(E[:], E[:], mybir.ActivationFunctionType.Exp)
            wg = _seg_icols(w) * PB   # num_idxs must be a multiple of 16
            nc.gpsimd.ap_gather(
                ETP[:, g:g + wg, :], E[:].rearrange("p g c -> p (g c)"),
                ICI[:, icol:icol + _seg_icols(w)],
                channels=P, num_elems=w * C // 2, d=2, num_idxs=wg,
            )
            EH1 = hpool.tile([P, w, CH], bf16, tag="EH1")
            nc.vector.tensor_add(EH1[:], E[:, :, 0:CH], E[:, :, CH:C])
            EH2 = hpool.tile([P, w, CH // 2], bf16, tag="EH2")
            nc.vector.tensor_add(EH2[:], EH1[:, :, 0:CH // 2], EH1[:, :, CH // 2:CH])
            # third fold: 25 -> 13 (12 pairs + the odd column copied across)
            EH3 = hpool.tile([P, w, 13], bf16, tag="EH3")
            nc.vector.tensor_add(EH3[:, :, 0:12], EH2[:, :, 0:12], EH2[:, :, 12:24])
            nc.vector.tensor_copy(EH3[:, :, 12], EH2[:, :, 24])
            nc.vector.reduce_sum(SE[:, g:g + w], EH3[:], axis=X)
            g += w
            icol += _seg_icols(w)
            # packed tails: coarse (4 matmul-tiles) through the steady
            # stream, segment-fine near the end so the final Ln/DVE chain
            # overlaps the last exps instead of serializing after them
            fine = g > G_ALL - 3 * GT
            if (g % CHUNK_G == 0) or (fine and (g % GT == 0 or g == G_ALL)):
                chunk_smalls(g_small_done, g)
                g_small_done = g
            if g % GT == 0 and (g % CHUNK_G == 0 or fine):
                chunk_mm(t_mm_done, g // GT)
                t_mm_done = g // GT

        osb = pk.tile([12, T * NQ], f32)
        nc.vector.tensor_copy(osb[:], psum[:])
        nc.scalar.dma_start(out=out_d, in_=osb[:])

    nc.compile()
    _GRAPH_CACHE[T] = nc
    return nc


CHUNK_G = GT       # g-slots per packed-DVE chunk (1 matmul-tile)


def _host_prep(logits, target):
    """Class-sorted block sharding. Returns per-core device inputs plus the
    block->class map for the host-side reduction."""
    N = target.shape[0]
    counts = np.bincount(target, minlength=C).astype(np.int64)
    order = np.argsort(target, kind="stable").astype(np.int64)

    nb_per_class = np.where(counts > 0, (counts + BLOCK - 1) // BLOCK, 0)
    B = int(nb_per_class.sum())
    T = max(1, math.ceil(B / (CORES * BPT)))
    Bcap = CORES * T * BPT

    row_src = np.zeros(Bcap * BLOCK, np.int64)
    valid = np.zeros(Bcap * BLOCK, np.float32)
    bcls = np.zeros(Bcap, np.int64)

    pos = 0
    b = 0
    for c in range(C):
        cnt = int(counts[c])
        if cnt == 0:
            continue
        nb = int(nb_per_class[c])
        row_src[b * BLOCK: b * BLOCK + cnt] = order[pos:pos + cnt]
        valid[b * BLOCK: b * BLOCK + cnt] = 1.0
        bcls[b:b + nb] = c
        pos += cnt
        b += nb
    assert pos == N and b == B

    # [core, t, q, gb, i, j] -> partition p = 16q + i, slot g = t*GT + gb*GB + j
    rs = row_src.reshape(CORES, T, NQ, NGB, PB, GB)
    idx = rs.transpose(0, 2, 4, 1, 3, 5).reshape(CORES, P, T * GT)
    tcls = bcls.reshape(CORES, T, NQ, NGB)
    # rows beyond each class's real count duplicate global row 0; the host
    # subtracts their contributions (device-mimicked) from the class sums
    npad = (BLOCK - valid.reshape(Bcap, BLOCK).sum(1)).astype(np.int64)

    # ap_gather PAIR indices (bf16, d=2), wrapped per SEGMENT: within a
    # segment starting at g-offset go with width w, out slot k reads the
    # pair (k*C + class(q, (go+k)//GB)) // 2; stored at
    # [16q + k%16, icol + k//16].  The parity plane selects the half.
    segs = _segments(T)
    NC_TOT = sum(_seg_icols(w) for w in segs)
    ic = np.zeros((CORES, P, NC_TOT), np.int16)
    par = np.zeros((CORES, P, T * GT), np.uint8)
    go = 0
    icol = 0
    for w in segs:
        k = np.arange(w)
        for q in range(NQ):
            # class of g-slot go+k for partition-group q: [core, w]
            cls_q = np.repeat(tcls[:, :, q, :].reshape(CORES, T * NGB),
                              GB, axis=1)[:, go:go + w]
            # adjacent advanced indices stay in place: result is [CORES, w]
            v = ((k[None, :] * C + cls_q) // 2).astype(np.int16)
            ic[:, PB * q + (k % PB), icol + k // PB] = v
            par[:, PB * q:PB * (q + 1), go:go + w] = \
                (cls_q % 2).astype(np.uint8)[:, None, :]
        go += w
        icol += _seg_icols(w)

    blockid = (np.arange(P)[:, None] // PB == np.arange(NQ)[None, :]).astype(np.float32)

    lg_np_dt = ml_dtypes.bfloat16 if LOGITS_BF16 else np.float32
    in_maps = []
    for core in range(CORES):
        lg_core = np.ascontiguousarray(
            logits[idx[core].reshape(-1)].reshape(P, T * GT * C).astype(lg_np_dt))
        in_maps.append({
            "logits": lg_core,
            "parity": np.ascontiguousarray(par[core]),
            "icidx": np.ascontiguousarray(ic[core]),
            "blockid": blockid,
        })
    return T, in_maps, tcls, counts, npad, bcls


def _device_pt_row(logits_row):
    """Mimic the device's pt computation for one row (used to subtract the
    contributions of pad rows, which all duplicate global row 0)."""
    bf16 = ml_dtypes.bfloat16
    x = logits_row.astype(bf16) if LOGITS_BF16 else logits_row
    E = np.exp(np.asarray(x, np.float32)).astype(bf16)
    f = np.float32
    eh1 = (E[:CH].astype(f) + E[CH:].astype(f)).astype(bf16)
    eh2 = (eh1[:CH // 2].astype(f) + eh1[CH // 2:].astype(f)).astype(bf16)
    eh3 = np.concatenate([
        (eh2[:12].astype(f) + eh2[12:24].astype(f)).astype(bf16), eh2[24:25]])
    se = eh3.astype(f).sum(dtype=f)
    return (E.astype(f) * (f(1.0) / se)).astype(np.float64)


def _reduce_outputs(outs, tcls, counts, N, logits_row0, npad, bcls):
    """Per-block device partials -> per-class sums (pad rows subtracted) ->
    final scalar loss."""
    S = np.zeros((3, C), np.float64)
    for core in range(CORES):
        o = np.asarray(outs[core], np.float64).reshape(3, NGB, -1, NQ)
        ov = o.transpose(0, 2, 3, 1).reshape(3, -1)   # [v, (t, q, gb)]
        cls_flat = tcls[core].reshape(-1)             # (t, q, gb)
        for v in range(3):
            np.add.at(S[v], cls_flat, ov[v])

    # subtract the pad rows (all duplicates of global row 0)
    pt0 = _device_pt_row(logits_row0)
    npad_cls = np.zeros(C, np.float64)
    np.add.at(npad_cls, bcls, npad.astype(np.float64))
    S1 = S[0] - npad_cls * pt0
    S2 = S[1] - npad_cls * pt0 * pt0
    S3 = S[2] - npad_cls * np.log(pt0 + EPS)

    counts_f = counts.astype(np.float64)

    nz = counts_f > 0
    safe = np.where(nz, counts_f, 1.0)
    c_max = counts_f.max()
    alpha = np.where(nz, np.log(c_max / safe) + 1.0, 0.0)

    l1_mean = np.where(nz, (-S3) / safe, 1.0)
    loss1 = l1_mean * alpha

    p_avg = np.where(nz, S1 / safe, 1.0)
    var = (S2 - counts_f * p_avg * p_avg) / np.maximum(counts_f - 1.0, 1.0)
    var_safe = np.where(counts_f > 1, var, 1.0)
    p_std = np.where(counts_f > 1, np.sqrt(np.maximum(var_safe, 0.0)), 0.0)

    a = alpha - alpha.max()
    ea = np.exp(a)
    alpha_sm = ea / ea.sum()
    loss2_cls = p_std / p_avg * alpha_sm
    loss2_mean = float((counts_f * loss2_cls).sum()) / N

    return np.float32(loss1.mean() + loss2_mean)


def _run(logits, target, trace=False, trace_kwargs=None):
    logits = np.ascontiguousarray(np.asarray(logits, np.float32))
    target = np.asarray(target)
    if target.dtype not in (np.int32, np.int64):
        target = target.astype(np.int64)
    N = target.shape[0]

    T, in_maps, tcls, counts, npad, bcls = _host_prep(
        logits, target.astype(np.int64))
    nc = _build_graph(T)

    from concourse.bass_utils import run_bass_kernel_spmd
    res = run_bass_kernel_spmd(
        nc, in_maps, core_ids=list(range(CORES)), trace=trace,
        **(trace_kwargs or {}),
    )
    outs = [res.results[i]["out"] for i in range(CORES)]
    loss = _reduce_outputs(outs, tcls, counts, N, logits[0], npad, bcls)
    return loss, res


def kernel(logits, target):
    return _run(logits, target)[0]
